# revision 4
# baseline (speedup 1.0000x reference)
"""Trainium2 Bass kernel for nn_DGLJTNNEncoder (junction-tree GRU encoder).

Forest of B=512 perfect binary trees (depth 6, H=256, V=780). The reference
runs an edge-GRU message passing over up+down BFS levels, then a gather at
every node, returning h[roots]. Only the upward pass reaches the roots, so
the kernel computes just the upward levels, sharded 64 trees per NeuronCore
across 8 cores.

Layout: activations are fp16, column-major [H (2 partition slabs of 128), L].
Edges of each level are ordered so that the two child edges of level-d edge i
sit at positions i and i+L/2 of level d+1 -> predecessor sums are stride-1
adds. Leaf-edge messages are a pure vocab lookup:
m_leaf = sigmoid(emb@Wz_top+bz) * tanh(emb@Wh_top+bh), precomputed per vocab
entry into a DRAM table. Both the x embeddings and leaf messages are fetched
with SWDGE dma_gather(transpose=True): 512-index chunks of 512B fp16 rows
land directly in column-major SBUF slabs. All GRU matmuls run in fp16.
"""

import numpy as np

B, D, H, V = 512, 6, 256, 780
N_TREE = 127
NCORES = 8
T = B // NCORES                                  # 64 trees per core
LW = {d: T * (1 << d) for d in range(D + 1)}     # level width (edges) per core
NL = sum(LW[d] for d in range(D))                # 4032 non-leaf nodes
NLP = 4096                                       # padded to gather chunks
LF = LW[D]                                       # 4096 leaves
CH = 512                                         # gather chunk (s2m desc cap)


def _orders():
    ords = [np.stack([np.arange(T), np.zeros(T, np.int64)], 1)]
    for _ in range(D):
        t, c = ords[-1][:, 0], ords[-1][:, 1]
        ords.append(np.concatenate(
            [np.stack([t, 2 * c + 1], 1), np.stack([t, 2 * c + 2], 1)], 0))
    return ords


_ORDS = _orders()
_NL_NODE = np.concatenate([_ORDS[d] for d in (5, 4, 3, 2, 1, 0)], 0)  # [4032,(t,c)]
_LF_NODE = _ORDS[6]                                                   # [4096,(t,c)]
_OFF = {}
_o = 0
for _d in (5, 4, 3, 2, 1, 0):
    _OFF[_d] = _o
    _o += LW[_d]


def _wrap(idx):
    """[n] ints -> [128, n//16] int16 wrap layout (replicated per Q7 core)."""
    n = idx.shape[0]
    blk = idx.reshape(n // 16, 16).T.astype(np.int16)
    return np.ascontiguousarray(np.tile(blk, (8, 1)))


_NC_CACHE = None


def _build():
    import concourse.mybir as mybir
    import concourse.tile as tile
    from concourse import bacc
    from contextlib import ExitStack

    f32, f16, i16 = mybir.dt.float32, mybir.dt.float16, mybir.dt.int16
    AF, OP = mybir.ActivationFunctionType, mybir.AluOpType

    nc = bacc.Bacc("TRN2", num_swdge_queues=4, dynamic_dma_scratch_size=2048)
    emb = nc.dram_tensor("emb", [V, H], f32, kind="ExternalInput")
    Wz = nc.dram_tensor("Wz", [2 * H, H], f32, kind="ExternalInput")
    Wh = nc.dram_tensor("Wh", [2 * H, H], f32, kind="ExternalInput")
    Wr = nc.dram_tensor("Wr", [H, H], f32, kind="ExternalInput")
    Ur = nc.dram_tensor("Ur", [H, H], f32, kind="ExternalInput")
    Wg = nc.dram_tensor("Wg", [2 * H, H], f32, kind="ExternalInput")
    bz = nc.dram_tensor("bz", [H], f32, kind="ExternalInput")
    bh = nc.dram_tensor("bh", [H], f32, kind="ExternalInput")
    bur = nc.dram_tensor("bur", [H], f32, kind="ExternalInput")
    bg = nc.dram_tensor("bg", [H], f32, kind="ExternalInput")
    widn = nc.dram_tensor("widn", [128, NLP // 16], i16, kind="ExternalInput")
    widl = nc.dram_tensor("widl", [128, LF // 16], i16, kind="ExternalInput")
    ident = nc.dram_tensor("ident", [128, 128], f32, kind="ExternalInput")
    out = nc.dram_tensor("out", [H, T], f32, kind="ExternalOutput")

    with ExitStack() as ctx:
        tc = ctx.enter_context(tile.TileContext(nc))
        SB = ctx.enter_context(tc.tile_pool(name="sb", bufs=1))
        TP = ctx.enter_context(tc.tile_pool(name="tp", bufs=3))
        PS = ctx.enter_context(tc.tile_pool(name="ps", bufs=6, space="PSUM"))

        # ---- input DMAs ----
        def wload(name, src, kt):
            t = SB.tile([128, kt * H], f32, tag=name, name=name)
            nc.sync.dma_start(
                t[:].rearrange("p (k h) -> p k h", h=H),
                src[:].rearrange("(k p) h -> p k h", p=128))
            return t

        Wz_f = wload("Wzf", Wz, 4)
        Wh_f = wload("Whf", Wh, 4)
        Wr_f = wload("Wrf", Wr, 2)
        Ur_f = wload("Urf", Ur, 2)
        Wg_f = wload("Wgf", Wg, 4)

        def bload(name, src):
            t = SB.tile([128, 2], f32, tag=name)
            nc.sync.dma_start(t[:], src[:].rearrange("(m p) -> p m", p=128))
            return t

        bz_sb, bh_sb, bur_sb, bg_sb = (bload(n, s) for n, s in
                                       (("bz", bz), ("bh", bh), ("bur", bur), ("bg", bg)))
        bzr_f = SB.tile([1, H], f32, tag="bzr")
        nc.sync.dma_start(bzr_f[:1, :], bz[:].rearrange("(a h) -> a h", a=1))
        bhr_f = SB.tile([1, H], f32, tag="bhr")
        nc.sync.dma_start(bhr_f[:1, :], bh[:].rearrange("(a h) -> a h", a=1))
        id_sb = SB.tile([128, 128], f32, tag="ident", name="ident")
        nc.sync.dma_start(id_sb[:], ident[:])
        widn_sb = SB.tile([128, NLP // 16], i16, tag="widn", name="widn")
        nc.sync.dma_start(widn_sb[:], widn[:])
        widl_sb = SB.tile([128, LF // 16], i16, tag="widl", name="widl")
        nc.sync.dma_start(widl_sb[:], widl[:])
        est = SB.tile([128, 6 * H], f32, tag="est", name="est")
        nc.sync.dma_start(est[:].rearrange("p (t h) -> p t h", h=H),
                          emb[:768, :].rearrange("(t p) h -> p t h", p=128))
        etail = SB.tile([128, H], f32, tag="etail", name="etail")
        nc.sync.dma_start(etail[:12, :], emb[768:, :])

        # ---- fp16 conversions ----
        est16 = SB.tile([128, 6 * H], f16, tag="est16", name="est16")
        for t6 in range(6):
            nc.vector.tensor_copy(est16[:, H * t6:H * (t6 + 1)],
                                  est[:, H * t6:H * (t6 + 1)])
        etail16 = SB.tile([128, H], f16, tag="etail16", name="etail16")
        nc.vector.tensor_copy(etail16[:12, :], etail[:12, :])
        id16 = SB.tile([128, 128], f16, tag="id16", name="id16")
        nc.vector.tensor_copy(id16[:], id_sb[:])

        def conv16(name, src, kt):
            t = SB.tile([128, kt * H], f16, tag=name, name=name)
            for k in range(kt):
                nc.vector.tensor_copy(t[:, H * k:H * (k + 1)],
                                      src[:, H * k:H * (k + 1)])
            # stationary slices indexed [2*k + h2], plus the raw tile
            sl = [t[:, H * k + 128 * h2:H * k + 128 * (h2 + 1)]
                  for k in range(kt) for h2 in range(2)]
            return t, sl

        Wz16_t, Wz16 = conv16("Wz16", Wz_f, 4)
        Wh16_t, Wh16 = conv16("Wh16", Wh_f, 4)
        _, Wr16 = conv16("Wr16", Wr_f, 2)
        _, Ur16 = conv16("Ur16", Ur_f, 2)
        _, Wg16 = conv16("Wg16", Wg_f, 4)
        bzr16 = SB.tile([1, H], f16, tag="bzr16")
        nc.vector.tensor_copy(bzr16[:1, :], bzr_f[:1, :])
        bhr16 = SB.tile([1, H], f16, tag="bhr16")
        nc.vector.tensor_copy(bhr16[:1, :], bhr_f[:1, :])
        ones16 = SB.tile([1, 128], f16, tag="ones16")
        nc.vector.tensor_scalar_mul(ones16[:1, :], id16[:1, :], 0.0)
        nc.vector.tensor_scalar_add(ones16[:1, :], ones16[:1, :], 1.0)

        # ---- emb16 DRAM table + xA gathers ----
        embd = SB.tile([V, H], f16, space="DRAM", tag="embd", name="embd")
        nc.sync.dma_start(embd[:768, :].rearrange("(t p) h -> p t h", p=128),
                          est16[:].rearrange("p (t h) -> p t h", h=H))
        nc.sync.dma_start(embd[768:, :], etail16[:12, :])

        xA = []
        for c in range(NLP // CH):
            xt = SB.tile([128, 2 * CH], f16, tag=f"xA{c}", name=f"xA{c}")
            nc.gpsimd.dma_gather(
                xt[:].rearrange("p (j n) -> p j n", j=2),
                embd[:], widn_sb[:, c * CH // 16:(c + 1) * CH // 16],
                CH, CH, H, transpose=True, queue_num=c % 4)
            xA.append(xt)

        def xap(pos, w, k):
            c, base = pos // CH, pos % CH
            assert base + w <= CH
            return xA[c][:].rearrange("p (j n) -> p j n", j=2)[:, k, base:base + w]

        # ---- embT (column-major fp16 emb) via transposes ----
        embT = [SB.tile([128, 7 * 128], f16, tag=f"embT{k}", name=f"embT{k}")
                for k in range(2)]
        for ti in range(7):
            rows = 128 if ti < 6 else V - 768
            for k in range(2):
                src_ap = (est16[:, H * ti + 128 * k:H * ti + 128 * k + 128] if ti < 6
                          else etail16[:rows, 128 * k:128 * (k + 1)])
                pt = PS.tile([128, 1024], f16, tag="pst", name="pst", bufs=2)
                nc.tensor.transpose(pt[:, :rows], src_ap, id16[:rows, :rows])
                nc.scalar.copy(embT[k][:, 128 * ti:128 * ti + rows], pt[:, :rows])

        # ---- leaf message vocab table (vocab-major) -> Tmd DRAM ----
        Tmd = SB.tile([V, H], f16, space="DRAM", tag="Tmd", name="Tmd")
        for ti in range(7):
            rows = 128 if ti < 6 else V - 768
            vc = slice(128 * ti, 128 * ti + rows)
            pz = PS.tile([128, 512], f32, tag="ps", name="ps")
            ph = PS.tile([128, 512], f32, tag="ps", name="ps")
            for k in range(2):
                nc.tensor.matmul(pz[:rows, :H], embT[k][:, vc],
                                 Wz16_t[:, H * k:H * (k + 1)],
                                 start=(k == 0), stop=False)
            nc.tensor.matmul(pz[:rows, :H], ones16[:1, :rows], bzr16[:1, :],
                             start=False, stop=True)
            for k in range(2):
                nc.tensor.matmul(ph[:rows, :H], embT[k][:, vc],
                                 Wh16_t[:, H * k:H * (k + 1)],
                                 start=(k == 0), stop=False)
            nc.tensor.matmul(ph[:rows, :H], ones16[:1, :rows], bhr16[:1, :],
                             start=False, stop=True)
            zt = TP.tile([128, H], f16, tag="zt", name="zt", bufs=2)
            ht = TP.tile([128, H], f16, tag="ht", name="ht", bufs=2)
            nc.scalar.activation(zt[:rows, :], pz[:rows, :H], AF.Sigmoid)
            nc.scalar.activation(ht[:rows, :], ph[:rows, :H], AF.Tanh)
            tm = TP.tile([128, H], f16, tag="tm", name="tm", bufs=2)
            nc.vector.tensor_tensor(tm[:rows, :], zt[:rows, :], ht[:rows, :],
                                    op=OP.mult)
            nc.sync.dma_start(Tmd[128 * ti:128 * ti + rows, :], tm[:rows, :])

        # ---- mA gathers ----
        mA = []
        for c in range(LF // CH):
            mt = SB.tile([128, 2 * CH], f16, tag=f"mA{c}", name=f"mA{c}")
            nc.gpsimd.dma_gather(
                mt[:].rearrange("p (j n) -> p j n", j=2),
                Tmd[:], widl_sb[:, c * CH // 16:(c + 1) * CH // 16],
                CH, CH, H, transpose=True, queue_num=c % 4)
            mA.append(mt)

        def map_(c, k):
            return mA[c][:].rearrange("p (j n) -> p j n", j=2)[:, k, :]

        # ---- leaf level (d=6): s5 = pairsum(mA); r; arm5 = pairsum(r*mA) ----
        L5 = LW[5]
        s_nxt = [SB.tile([128, L5], f16, tag=f"s5_{k}", name=f"s5_{k}") for k in range(2)]
        arm_nxt = [SB.tile([128, L5], f16, tag=f"a5_{k}", name=f"a5_{k}") for k in range(2)]
        for c in range(4):
            for k in range(2):
                nc.vector.tensor_tensor(s_nxt[k][:, c * CH:(c + 1) * CH],
                                        map_(c, k), map_(c + 4, k), op=OP.add)
        for h2 in range(2):
            for ci in range(4):
                rms = []
                for half in range(2):
                    c = ci + 4 * half
                    pr = PS.tile([128, 512], f32, tag="ps", name="ps")
                    args = [(Wr16[0 + h2], xap(ci * CH, CH, 0)),
                            (Wr16[2 + h2], xap(ci * CH, CH, 1)),
                            (Ur16[0 + h2], map_(c, 0)),
                            (Ur16[2 + h2], map_(c, 1))]
                    for i, (w, rhs) in enumerate(args):
                        nc.tensor.matmul(pr[:], w, rhs, start=(i == 0), stop=(i == 3))
                    r_t = TP.tile([128, 512], f16, tag="rr", name="r_t", bufs=4)
                    nc.scalar.activation(r_t[:], pr[:], AF.Sigmoid,
                                         bias=bur_sb[:, h2:h2 + 1])
                    rmt = TP.tile([128, 512], f16, tag=f"rm{h2}", name="rmt", bufs=2)
                    nc.vector.tensor_tensor(rmt[:], r_t[:], map_(c, h2), op=OP.mult)
                    rms.append(rmt)
                nc.vector.tensor_tensor(arm_nxt[h2][:, ci * CH:(ci + 1) * CH],
                                        rms[0][:], rms[1][:], op=OP.add)

        # ---- levels d = 5..1 ----
        for d in (5, 4, 3, 2, 1):
            L = LW[d]
            s_cur, arm_cur = s_nxt, arm_nxt
            S = min(512, L)
            nsl = L // S
            s_nxt = [SB.tile([128, L // 2], f16, tag=f"s{d - 1}_{k}",
                             name=f"s{d - 1}_{k}") for k in range(2)]
            arm_nxt = ([SB.tile([128, L // 2], f16, tag=f"a{d - 1}_{k}",
                                name=f"a{d - 1}_{k}") for k in range(2)]
                       if d >= 2 else None)
            m_t = [TP.tile([128, L], f16, tag=f"m{d}_{k}", name="m_t", bufs=1)
                   for k in range(2)]
            for h2 in range(2):
                for sl in range(nsl):
                    cs = slice(sl * S, (sl + 1) * S)
                    pz = PS.tile([128, 512], f32, tag="ps", name="ps")
                    ph = PS.tile([128, 512], f32, tag="ps", name="ps")
                    argz = [(Wz16[0 + h2], xap(_OFF[d] + sl * S, S, 0)),
                            (Wz16[2 + h2], xap(_OFF[d] + sl * S, S, 1)),
                            (Wz16[4 + h2], s_cur[0][:, cs]),
                            (Wz16[6 + h2], s_cur[1][:, cs])]
                    for i, (w, rhs) in enumerate(argz):
                        nc.tensor.matmul(pz[:, :S], w, rhs, start=(i == 0), stop=(i == 3))
                    argh = [(Wh16[0 + h2], xap(_OFF[d] + sl * S, S, 0)),
                            (Wh16[2 + h2], xap(_OFF[d] + sl * S, S, 1)),
                            (Wh16[4 + h2], arm_cur[0][:, cs]),
                            (Wh16[6 + h2], arm_cur[1][:, cs])]
                    for i, (w, rhs) in enumerate(argh):
                        nc.tensor.matmul(ph[:, :S], w, rhs, start=(i == 0), stop=(i == 3))
                    z_t = TP.tile([128, 512], f16, tag="ew", name="z_t", bufs=6)
                    t_t = TP.tile([128, 512], f16, tag="ew", name="t_t", bufs=6)
                    nc.scalar.activation(z_t[:, :S], pz[:, :S], AF.Sigmoid,
                                         bias=bz_sb[:, h2:h2 + 1])
                    nc.scalar.activation(t_t[:, :S], ph[:, :S], AF.Tanh,
                                         bias=bh_sb[:, h2:h2 + 1])
                    dd = TP.tile([128, 512], f16, tag="ew", name="dd", bufs=6)
                    nc.vector.tensor_tensor(dd[:, :S], t_t[:, :S], s_cur[h2][:, cs],
                                            op=OP.subtract)
                    ee = TP.tile([128, 512], f16, tag="ew", name="ee", bufs=6)
                    nc.vector.tensor_tensor(ee[:, :S], z_t[:, :S], dd[:, :S], op=OP.mult)
                    nc.vector.tensor_tensor(m_t[h2][:, cs], ee[:, :S], s_cur[h2][:, cs],
                                            op=OP.add)
            if d >= 2:
                Sr = min(S, L // 2)
                for h2 in range(2):
                    for jh in range(0, L // 2, Sr):
                        rms = []
                        for half in range(2):
                            j0 = jh + half * (L // 2)
                            pp = _OFF[d - 1] + jh
                            pr = PS.tile([128, 512], f32, tag="ps", name="ps")
                            argr = [(Wr16[0 + h2], xap(pp, Sr, 0)),
                                    (Wr16[2 + h2], xap(pp, Sr, 1)),
                                    (Ur16[0 + h2], m_t[0][:, j0:j0 + Sr]),
                                    (Ur16[2 + h2], m_t[1][:, j0:j0 + Sr])]
                            for i, (w, rhs) in enumerate(argr):
                                nc.tensor.matmul(pr[:, :Sr], w, rhs,
                                                 start=(i == 0), stop=(i == 3))
                            r_t = TP.tile([128, 512], f16, tag="rr", name="r_t", bufs=4)
                            nc.scalar.activation(r_t[:, :Sr], pr[:, :Sr], AF.Sigmoid,
                                                 bias=bur_sb[:, h2:h2 + 1])
                            rmt = TP.tile([128, 512], f16, tag=f"lrm{h2}",
                                          name="rmt", bufs=2)
                            nc.vector.tensor_tensor(rmt[:, :Sr], r_t[:, :Sr],
                                                    m_t[h2][:, j0:j0 + Sr], op=OP.mult)
                            rms.append(rmt)
                        nc.vector.tensor_tensor(arm_nxt[h2][:, jh:jh + Sr],
                                                rms[0][:, :Sr], rms[1][:, :Sr],
                                                op=OP.add)
            for h2 in range(2):
                nc.vector.tensor_tensor(s_nxt[h2][:], m_t[h2][:, :L // 2],
                                        m_t[h2][:, L // 2:], op=OP.add)

        # ---- roots: h = relu(Wg^T [x_root; node_m] + bg) ----
        for h2 in range(2):
            pg = PS.tile([128, 512], f32, tag="ps", name="ps")
            argg = [(Wg16[0 + h2], xap(_OFF[0], T, 0)),
                    (Wg16[2 + h2], xap(_OFF[0], T, 1)),
                    (Wg16[4 + h2], s_nxt[0][:]),
                    (Wg16[6 + h2], s_nxt[1][:])]
            for i, (w, rhs) in enumerate(argg):
                nc.tensor.matmul(pg[:, :T], w, rhs, start=(i == 0), stop=(i == 3))
            o_t = SB.tile([128, T], f32, tag=f"o{h2}", name=f"o{h2}")
            nc.scalar.activation(o_t[:], pg[:, :T], AF.Relu, bias=bg_sb[:, h2:h2 + 1])
            nc.sync.dma_start(out[128 * h2:128 * (h2 + 1), :], o_t[:])

    nc.compile()
    return nc


def _prep(inputs):
    global _NC_CACHE

    wid = np.asarray(inputs["wid"]).astype(np.int64)
    base = {k: np.ascontiguousarray(np.asarray(inputs[k], np.float32))
            for k in ("emb", "Wz", "bz", "Wr", "Ur", "bur", "Wh", "bh", "Wg", "bg")}
    base["ident"] = np.eye(128, dtype=np.float32)

    in_maps = []
    pad = np.zeros(NLP - NL, np.int64)
    for c in range(NCORES):
        nn = (c * T + _NL_NODE[:, 0]) * N_TREE + _NL_NODE[:, 1]
        ln = (c * T + _LF_NODE[:, 0]) * N_TREE + _LF_NODE[:, 1]
        in_maps.append({**base,
                        "widn": _wrap(np.concatenate([wid[nn], pad])),
                        "widl": _wrap(wid[ln])})

    if _NC_CACHE is None:
        _NC_CACHE = _build()
    return _NC_CACHE, in_maps


def kernel(**inputs):
    from concourse import bass_utils

    nc, in_maps = _prep(inputs)
    res = bass_utils.run_bass_kernel_spmd(nc, in_maps, core_ids=list(range(NCORES)))
    outs = [np.ascontiguousarray(np.asarray(r["out"]).T) for r in res.results]
    return np.concatenate(outs, 0).astype(np.float32)


if __name__ == "__main__":
    rng = np.random.default_rng(0)
    ins = {
        "wid": rng.integers(0, V, B * N_TREE).astype(np.int32),
        "emb": rng.standard_normal((V, H), dtype=np.float32),
        "Wz": rng.standard_normal((2 * H, H), dtype=np.float32) / 22.6,
        "bz": rng.standard_normal(H).astype(np.float32),
        "Wr": rng.standard_normal((H, H), dtype=np.float32) / 16.0,
        "Ur": rng.standard_normal((H, H), dtype=np.float32) / 16.0,
        "bur": rng.standard_normal(H).astype(np.float32),
        "Wh": rng.standard_normal((2 * H, H), dtype=np.float32) / 22.6,
        "bh": rng.standard_normal(H).astype(np.float32),
        "Wg": rng.standard_normal((2 * H, H), dtype=np.float32) / 22.6,
        "bg": rng.standard_normal(H).astype(np.float32),
    }
    o = kernel(**ins)
    print("kernel output", o.shape, o.dtype, float(np.abs(o).max()))
